# revision 17
# baseline (speedup 1.0000x reference)
"""Trainium2 Bass kernel for BasicIcoS2SUpBlock (upsample + hex-conv ×3 + sync-BN ×3).

Strategy
--------
Data parallel over batch: B=16 -> 2 samples per core on 8 NeuronCores.
Sync-BN via two tiny AllReduce collectives of per-channel (sum, sumsq).

The nearest-neighbor 2x upsample is folded into conv00/conv10: each output
parity class (a, c) in {0,1}^2 sees an effective 2x2 kernel on the
*unsampled* input x, so the convs run on the [160, 64] grid instead of
[320, 128] (43% fewer MACs, no materialized upsample).

All tensors on chip live in a "parity layout": partition p = ch + 32*(2a+c),
free dim = (i, j) on the [160, 64] parity grid. conv01 (the second hex conv,
on the full-resolution h) becomes, per output parity, a sum of 4 matmuls
whose contraction dim K = 32 channels x 4 input parity groups = 128 - the
parity groups are already stacked in partitions, so no data duplication.

Matmuls: K=128, M=32, N=512, 4-way column tiling via tile_position so the
four parity groups of one output tile run concurrently in the PE array.
conv00/conv10 use float32r (TF32-like, full rate, no cast pass from the f32
input); conv01 runs in bf16 (h is produced in bf16 by the scalar engine).
"""

import sys

for _p in ("/opt/trn_rl_repo", "/root/.axon_site/_ro/trn_rl_repo"):
    if _p not in sys.path:
        sys.path.insert(0, _p)

import numpy as np
import ml_dtypes

import concourse.bass as bass
import concourse.bacc as bacc
import concourse.mybir as mybir
import concourse.tile as tile
from concourse import bass_utils

F32 = mybir.dt.float32
F32R = mybir.dt.float32r
BF16 = mybir.dt.bfloat16
ALU = mybir.AluOpType
ACT = mybir.ActivationFunctionType
AX = mybir.AxisListType

N_CORES = 8
B, CIN, COUT = 16, 64, 32
HX, WX = 160, 64          # x spatial (per sample); parity grid is the same size
B_LOC = B // N_CORES      # 2 samples per core
NCH = HX // 8             # 20 chunks of 8 parity-grid rows per sample
NTILE = B_LOC * NCH       # 40 [128, 512] tiles per intermediate tensor
CPX = 512                 # elements per chunk per partition (8 rows x 64 cols)
HW2 = (HX + 2) * (WX + 2)  # 162*66 halo-padded plane, per partition
BN_EPS = 1e-5
N_STAT = float(B * 2 * HX * 2 * WX)  # BN stat count per channel: 16*320*128
NSLOT = 96

_HEX = np.ones((3, 3), np.float32)
_HEX[0, 2] = 0.0
_HEX[2, 0] = 0.0

_A = {0: np.array([[1, 0, 0], [0, 1, 1]], np.float32),
      1: np.array([[1, 1, 0], [0, 0, 1]], np.float32)}


def _fold_up(w):
    """w [Cout, Cin, 3, 3] -> lhsT blocks [8, 128, 32], idx = a*4 + c*2 + u.

    Block rows: ci + 64*v (v = column-shift tap), cols: co.
    """
    wt = w * _HEX
    out = np.zeros((8, 128, 32), np.float32)
    for a in range(2):
        for c in range(2):
            e = np.einsum('ud,ve,oide->uvoi', _A[a], _A[c], wt)  # [2,2,Cout,Cin]
            for u in range(2):
                blk = np.zeros((128, 32), np.float32)
                for v in range(2):
                    blk[64 * v:64 * v + 64, :] = e[u, v].T
                out[a * 4 + c * 2 + u] = blk
    return out


def _fold_c01(w):
    """w01 [Cout, 32, 3, 3] -> lhsT blocks [16, 128, 32], idx = al*8+ga*4+d*2+e.

    Block rows: ci + 32*(2*a' + c'), cols: co.
    """
    wt = w * _HEX
    wd = np.zeros((2, 2, 2, 2, 2, 2, w.shape[0], w.shape[1]), np.float32)
    for al in range(2):
        for dy in range(3):
            ap_ = (al + dy - 1) % 2
            d = (al + dy - 1) // 2 - (al - 1)
            for ga in range(2):
                for dx in range(3):
                    cp_ = (ga + dx - 1) % 2
                    e = (ga + dx - 1) // 2 - (ga - 1)
                    wd[al, ga, d, e, ap_, cp_] += wt[:, :, dy, dx]
    out = np.zeros((16, 128, 32), np.float32)
    for al in range(2):
        for ga in range(2):
            for d in range(2):
                for e in range(2):
                    blk = np.zeros((128, 32), np.float32)
                    for ap_ in range(2):
                        for cp_ in range(2):
                            g = 2 * ap_ + cp_
                            blk[32 * g:32 * g + 32, :] = wd[al, ga, d, e, ap_, cp_].T
                    out[al * 8 + ga * 4 + d * 2 + e] = blk
    return out


def _emit(nc, tc, x_in, w00l, w10l, w01l, gb, out):
    with (
        tc.tile_pool(name="setup", bufs=1) as setup,
        tc.tile_pool(name="dram", bufs=1, space="DRAM") as dram,
        tc.tile_pool(name="stores", bufs=1) as stores,
        tc.tile_pool(name="scrap", bufs=3) as scrap,
        tc.tile_pool(name="ps", bufs=6, space="PSUM") as ps,
    ):
        # --- weights / gamma-beta ---
        wu00 = setup.tile([128, 8 * 32], BF16)
        wu10 = setup.tile([128, 8 * 32], BF16)
        w01t = setup.tile([128, 16 * 32], BF16)
        nc.sync.dma_start(wu00[:].rearrange("p (b c) -> p b c", b=8),
                          w00l.rearrange("b p c -> p b c"))
        nc.sync.dma_start(wu10[:].rearrange("p (b c) -> p b c", b=8),
                          w10l.rearrange("b p c -> p b c"))
        nc.sync.dma_start(w01t[:].rearrange("p (b c) -> p b c", b=16),
                          w01l.rearrange("b p c -> p b c"))
        gbt = setup.tile([32, 6], F32)  # (g00,b00,g01,b01,g10,b10) per channel
        nc.sync.dma_start(gbt[:], gb.rearrange("t c -> c t"))

        # --- stats buffers ---
        ssum = [setup.tile([128, NSLOT], F32, tag=f"ssum{t}", name=f"ssum{t}") for t in range(3)]
        ssq = [setup.tile([128, NSLOT], F32, tag=f"ssq{t}", name=f"ssq{t}") for t in range(3)]
        pp = [setup.tile([128, 2], F32, tag=f"pp{t}", name=f"pp{t}") for t in range(3)]
        dpp = [dram.tile([128, 2], F32, tag=f"dpp{t}", name=f"dpp{t}") for t in range(3)]
        gath = [setup.tile([32, 8], F32, tag=f"gath{t}", name=f"gath{t}") for t in range(3)]
        loc = [setup.tile([32, 2], F32, tag=f"loc{t}", name=f"loc{t}") for t in range(3)]

        ar1_in = dram.tile([32, 2], F32, tag="ar1i")
        ar1_out = dram.tile([32, 2], F32, tag="ar1o")
        ar2_in = dram.tile([32, 4], F32, tag="ar2i")
        ar2_out = dram.tile([32, 4], F32, tag="ar2o")

        sg1 = setup.tile([32, 2], F32)   # AR1 result (c00: sum, sumsq)
        sg2 = setup.tile([32, 4], F32)   # AR2 result (c10 | c01)

        st0 = setup.tile([32, 2], F32)   # (scale0, shift0) for BN0
        st0r = setup.tile([128, 2], F32)
        fin = setup.tile([32, 3], F32)   # (r = s2/s1, s1, b1+b2)
        finr = setup.tile([128, 3], F32)
        sm = setup.tile([32, 12], F32)   # small-math scratch
        eps = setup.tile([32, 1], F32)
        nc.gpsimd.memset(eps[:], BN_EPS)

        # --- intermediate stores (bf16, parity layout) ---
        c00s = stores.tile([128, NTILE * CPX], BF16, tag="c00")
        c10s = stores.tile([128, NTILE * CPX], BF16, tag="c10")

        def _copy_evac(dst, src_ap, t, slot):
            """PSUM -> bf16 store copy with free per-partition sum accumulation."""
            if slot % 3 == 2:  # 1/3 of evacuations on the vector engine
                nc.vector.tensor_scalar(dst, src_ap, 1.0, 0.0, ALU.mult,
                                        ALU.add,
                                        accum_out=ssum[t][:, slot:slot + 1])
            else:
                nc.scalar.activation(dst, src_ap, ACT.Copy,
                                     accum_out=ssum[t][:, slot:slot + 1])

        def _sumsq(store, idx, t, slot):
            dst = store[:, idx * CPX:(idx + 1) * CPX]
            sq = scrap.tile([128, CPX], BF16, tag="sqscrap")
            nc.vector.scalar_tensor_tensor(
                sq[:], dst, 1.0, dst, ALU.mult, ALU.mult,
                accum_out=ssq[t][:, slot:slot + 1])

        def _evac(pt, store, t, idx):
            """Parity-layout evacuation (c00): one [128, 512] copy."""
            dst = store[:, idx * CPX:(idx + 1) * CPX]
            _copy_evac(dst, pt[:], t, idx)
            _sumsq(store, idx, t, idx)

        def _evac2(pt0, pt1, store, t, idx):
            """W-interleaving evacuation (c10/c01): two [128, 256] strided copies.

            pt0/pt1 hold the c=0 / c=1 W-parity halves; partition groups are
            (a, row-half); free layout becomes (i 4, W 128) with W = 2j + c.
            """
            dstv = store[:, idx * CPX:(idx + 1) * CPX].rearrange(
                "p (i j c) -> p i j c", i=4, c=2)
            _copy_evac(dstv[:, :, :, 0], pt0[:], t, 2 * idx)
            _copy_evac(dstv[:, :, :, 1], pt1[:], t, 2 * idx + 1)
            _sumsq(store, idx, t, idx)

        def stats_combine(t, dst_ap, nsum=NTILE, nsq=NTILE):
            nc.vector.reduce_sum(pp[t][:, 0:1], ssum[t][:, 0:nsum], axis=AX.X)
            nc.vector.reduce_sum(pp[t][:, 1:2], ssq[t][:, 0:nsq], axis=AX.X)
            nc.sync.dma_start(dpp[t][:], pp[t][:])
            nc.sync.dma_start(gath[t][:].rearrange("p (g i) -> p g i", g=4),
                              dpp[t][:].rearrange("(g p) i -> p g i", p=32))
            g = gath[t]
            nc.vector.tensor_add(g[:, 0:2], g[:, 0:2], g[:, 2:4])
            nc.vector.tensor_add(g[:, 4:6], g[:, 4:6], g[:, 6:8])
            nc.vector.tensor_add(loc[t][:], g[:, 0:2], g[:, 4:6])
            nc.sync.dma_start(dst_ap, loc[t][:])

        def bn_coeffs(stats2, gamma_ap, beta_ap, s_dst, t_dst):
            """stats2 [32,2]=(sum,sumsq) -> BN scale/shift into s_dst/t_dst."""
            mv = sm[:, 0:2]
            nc.vector.tensor_scalar(mv, stats2, 1.0 / N_STAT, None, ALU.mult)
            m2 = sm[:, 2:3]
            nc.vector.tensor_mul(m2, mv[:, 0:1], mv[:, 0:1])
            var = sm[:, 3:4]
            nc.vector.tensor_sub(var, mv[:, 1:2], m2)
            std = sm[:, 2:3]  # reuse
            nc.scalar.activation(std, var, ACT.Sqrt, bias=eps[:])
            rinv = sm[:, 3:4]  # reuse
            nc.vector.reciprocal(rinv, std)
            nc.vector.tensor_mul(s_dst, rinv, gamma_ap)
            msc = sm[:, 2:3]
            nc.vector.tensor_mul(msc, mv[:, 0:1], s_dst)
            nc.vector.tensor_sub(t_dst, beta_ap, msc)

        # ================= phase A: conv00 / conv10 ==========================
        with tc.tile_pool(name="x2", bufs=2) as x2p:
            x2 = []
            for s in range(B_LOC):
                t = x2p.tile([128, HW2], BF16, tag="x2")
                v = t[:].rearrange("p (r c) -> p r c", c=WX + 2)
                # zero borders: orig half cols {0, 65}; dup half cols {64, 65}
                nc.gpsimd.memset(v[0:64, :, 0:1], 0.0)
                nc.gpsimd.memset(v[0:64, :, WX + 1:WX + 2], 0.0)
                nc.gpsimd.memset(v[64:128, :, WX:WX + 2], 0.0)
                rb = HX // 4
                for q in range(4):
                    src = x_in[s, :, q * rb:(q + 1) * rb, :]
                    nc.sync.dma_start(
                        v[0:64, 1 + q * rb:1 + (q + 1) * rb, 1:WX + 1], src)
                    nc.sync.dma_start(
                        v[64:128, 1 + q * rb:1 + (q + 1) * rb, 0:WX], src)
                # wrap halo rows
                nc.sync.dma_start(v[0:64, 0:1, 1:WX + 1], x_in[s, :, HX - 1:HX, :])
                nc.sync.dma_start(v[0:64, HX + 1:HX + 2, 1:WX + 1], x_in[s, :, 0:1, :])
                nc.sync.dma_start(v[64:128, 0:1, 0:WX], x_in[s, :, HX - 1:HX, :])
                nc.sync.dma_start(v[64:128, HX + 1:HX + 2, 0:WX], x_in[s, :, 0:1, :])
                x2.append(v)

            def upconv_parity(wt, store, tensor_idx):
                """Parity-layout up-conv (c00): groups (a, c), N=512."""
                for s in range(B_LOC):
                    for k in range(NCH):
                        idx = s * NCH + k
                        pt = ps.tile([128, CPX], F32, tag="ps", bufs=3)
                        for g in range(4):
                            a, c = g >> 1, g & 1
                            for u in range(2):
                                widx = a * 4 + c * 2 + u
                                rhs = x2[s][:, 8 * k + u + a:8 * k + u + a + 8,
                                            c:c + WX]
                                nc.tensor.matmul(
                                    pt[32 * g:32 * g + 32, :],
                                    wt[:, widx * 32:(widx + 1) * 32],
                                    rhs, start=(u == 0), stop=(u == 1),
                                    tile_position=(0, 32 * g))
                        _evac(pt, store, tensor_idx, idx)

            def upconv_il(wt, store, tensor_idx):
                """W-interleaved up-conv (c10): groups (a, row-half), N=256."""
                for s in range(B_LOC):
                    for k in range(NCH):
                        idx = s * NCH + k
                        pts = [ps.tile([128, CPX // 2], F32, tag=f"psh{c}",
                                       bufs=2, name=f"psh{c}")
                               for c in range(2)]
                        for c in range(2):
                            for g in range(4):
                                a, ih = g >> 1, g & 1
                                r0 = 8 * k + 4 * ih
                                for u in range(2):
                                    widx = a * 4 + c * 2 + u
                                    rhs = x2[s][:, r0 + u + a:r0 + u + a + 4,
                                                c:c + WX]
                                    nc.tensor.matmul(
                                        pts[c][32 * g:32 * g + 32, :],
                                        wt[:, widx * 32:(widx + 1) * 32],
                                        rhs, start=(u == 0), stop=(u == 1),
                                        tile_position=(0, 32 * g))
                        _evac2(pts[0], pts[1], store, tensor_idx, idx)

            upconv_parity(wu00, c00s, 0)
            stats_combine(0, ar1_in[:])
            nc.gpsimd.collective_compute(
                "AllReduce", ALU.add, replica_groups=[list(range(N_CORES))],
                ins=[ar1_in.opt()], outs=[ar1_out.opt()])
            nc.sync.dma_start(sg1[:], ar1_out[:])
            bn_coeffs(sg1[:], gbt[:, 0:1], gbt[:, 1:2], st0[:, 0:1], st0[:, 1:2])
            for g in range(4):
                nc.sync.dma_start(st0r[32 * g:32 * g + 32, :], st0[:])

            upconv_il(wu10, c10s, 1)  # overlaps AllReduce #1
            stats_combine(1, ar2_in[:, 0:2], nsum=2 * NTILE, nsq=NTILE)

        # ================= phase B: h = relu(BN0(c00)); conv01 ===============
        with (
            tc.tile_pool(name="h", bufs=2) as hp,
            tc.tile_pool(name="c01p", bufs=1) as c01p,
        ):
            c01s = c01p.tile([128, NTILE * CPX], BF16, tag="c01")
            for s in range(B_LOC):
                ht = hp.tile([128, HW2], BF16, tag="h")
                hv = ht[:].rearrange("p (r c) -> p r c", c=WX + 2)
                nc.gpsimd.memset(hv[:, :, 0:1], 0.0)
                nc.gpsimd.memset(hv[:, :, WX + 1:WX + 2], 0.0)
                for k in [NCH - 1] + list(range(NCH - 1)):
                    idx = s * NCH + k
                    src = c00s[:, idx * CPX:(idx + 1) * CPX].rearrange(
                        "p (r c) -> p r c", c=WX)
                    nc.scalar.activation(hv[:, 8 * k + 1:8 * k + 9, 1:WX + 1],
                                         src, ACT.Relu, scale=st0r[:, 0:1],
                                         bias=st0r[:, 1:2])
                # wrap halo rows (within each parity group)
                nc.vector.tensor_copy(hv[:, 0:1, :], hv[:, HX:HX + 1, :])
                nc.vector.tensor_copy(hv[:, HX + 1:HX + 2, :], hv[:, 1:2, :])

                for k in range(NCH):
                    idx = s * NCH + k
                    pts = [ps.tile([128, CPX // 2], F32, tag=f"psh{c}",
                                   bufs=2, name=f"psh{c}")
                           for c in range(2)]
                    for ga in range(2):
                        for m in range(4):
                            al, ih = m >> 1, m & 1
                            r0 = 8 * k + 4 * ih
                            first = True
                            for d in range(2):
                                for e in range(2):
                                    widx = al * 8 + ga * 4 + d * 2 + e
                                    rhs = hv[:, r0 + d + al:r0 + d + al + 4,
                                             e + ga:e + ga + WX]
                                    nc.tensor.matmul(
                                        pts[ga][32 * m:32 * m + 32, :],
                                        w01t[:, widx * 32:(widx + 1) * 32],
                                        rhs, start=first,
                                        stop=(d == 1 and e == 1),
                                        tile_position=(0, 32 * m))
                                    first = False
                    _evac2(pts[0], pts[1], c01s, 2, idx)

            stats_combine(2, ar2_in[:, 2:4], nsum=2 * NTILE, nsq=NTILE)
            nc.gpsimd.collective_compute(
                "AllReduce", ALU.add, replica_groups=[list(range(N_CORES))],
                ins=[ar2_in.opt()], outs=[ar2_out.opt()])
            nc.sync.dma_start(sg2[:], ar2_out[:])

            # BN1 (c01, g01/b01) and BN2 (c10, g10/b10)
            s1, t1 = sm[:, 4:5], sm[:, 5:6]
            s2, t2 = sm[:, 6:7], sm[:, 7:8]
            bn_coeffs(sg2[:, 2:4], gbt[:, 2:3], gbt[:, 3:4], s1, t1)
            bn_coeffs(sg2[:, 0:2], gbt[:, 4:5], gbt[:, 5:6], s2, t2)
            rs1 = sm[:, 8:9]
            nc.vector.reciprocal(rs1, s1)
            nc.vector.tensor_mul(fin[:, 0:1], s2, rs1)   # r = s2/s1
            nc.vector.tensor_copy(fin[:, 1:2], s1)
            nc.vector.tensor_add(fin[:, 2:3], t1, t2)    # b' = t1 + t2
            for g in range(4):
                nc.sync.dma_start(finr[32 * g:32 * g + 32, :], fin[:])

            # ============ phase C: out = relu(s1*(c01 + r*c10) + b') =========
            # stores are W-interleaved: partition ch + 32*(2a + ih),
            # free (i 4, W 128); out row H = 2*(8k + 4*ih + i) + a.
            ov = out.rearrange("s ch (i2 a) w -> s a ch i2 w", a=2)
            for s in range(B_LOC):
                for k in range(NCH):
                    idx = s * NCH + k
                    tmp = scrap.tile([128, CPX], BF16, tag="fintmp")
                    nc.vector.scalar_tensor_tensor(
                        tmp[:], c10s[:, idx * CPX:(idx + 1) * CPX],
                        finr[:, 0:1], c01s[:, idx * CPX:(idx + 1) * CPX],
                        ALU.mult, ALU.add)
                    ot = scrap.tile([128, CPX], F32, tag="finout")
                    nc.scalar.activation(ot[:], tmp[:], ACT.Relu,
                                         scale=finr[:, 1:2], bias=finr[:, 2:3])
                    for g in range(4):
                        a, ih = g >> 1, g & 1
                        dst = ov[s, a][:, 8 * k + 4 * ih:8 * k + 4 * ih + 4, :]
                        src = ot[32 * g:32 * g + 32, :].rearrange(
                            "ch (i w) -> ch i w", w=2 * WX)
                        nc.sync.dma_start(dst, src)


def _build_nc():
    nc = bacc.Bacc("TRN2", target_bir_lowering=False, debug=False,
                   num_devices=N_CORES)
    x_in = nc.dram_tensor("x", [B_LOC, CIN, HX, WX], BF16,
                          kind="ExternalInput").ap()
    w00l = nc.dram_tensor("w00l", [8, 128, 32], BF16, kind="ExternalInput").ap()
    w10l = nc.dram_tensor("w10l", [8, 128, 32], BF16, kind="ExternalInput").ap()
    w01l = nc.dram_tensor("w01l", [16, 128, 32], BF16, kind="ExternalInput").ap()
    gb = nc.dram_tensor("gb", [6, 32], F32, kind="ExternalInput").ap()
    out = nc.dram_tensor("out", [B_LOC, COUT, 2 * HX, 2 * WX], F32,
                         kind="ExternalOutput").ap()
    with tile.TileContext(nc) as tc:
        _emit(nc, tc, x_in, w00l, w10l, w01l, gb, out)
    nc.compile()
    return nc


_CACHE = {}


def _get_nc():
    if "nc" not in _CACHE:
        _CACHE["nc"] = _build_nc()
    return _CACHE["nc"]


def _make_in_maps(inputs):
    x = np.ascontiguousarray(
        np.asarray(inputs["x"], dtype=np.float32).astype(ml_dtypes.bfloat16))
    gb = np.stack([np.asarray(inputs[k], dtype=np.float32)
                   for k in ("g00", "b00", "g01", "b01", "g10", "b10")])
    w00l = _fold_up(np.asarray(inputs["w00"], dtype=np.float32)).astype(
        ml_dtypes.bfloat16)
    w10l = _fold_up(np.asarray(inputs["w10"], dtype=np.float32)).astype(
        ml_dtypes.bfloat16)
    w01l = _fold_c01(np.asarray(inputs["w01"], dtype=np.float32)).astype(
        ml_dtypes.bfloat16)
    return [{"x": x[i * B_LOC:(i + 1) * B_LOC],
             "w00l": w00l, "w10l": w10l, "w01l": w01l, "gb": gb}
            for i in range(N_CORES)]


def kernel(**inputs) -> np.ndarray:
    in_maps = _make_in_maps(inputs)
    nc = _get_nc()
    res = bass_utils.run_bass_kernel_spmd(nc, in_maps,
                                          core_ids=list(range(N_CORES)))
    return np.concatenate([r["out"] for r in res.results], axis=0)


# revision 20
# speedup vs baseline: 14364.9599x; 14364.9599x over previous
"""Trainium2 Bass kernel for BasicIcoS2SUpBlock (upsample + hex-conv ×3 + sync-BN ×3).

Strategy
--------
Data parallel over batch: B=16 -> 2 samples per core on 8 NeuronCores.
Sync-BN via two tiny AllReduce collectives of per-channel (sum, sumsq).

The nearest-neighbor 2x upsample is folded into conv00/conv10: each output
parity class (a, c) in {0,1}^2 sees an effective 2x2 kernel on the
*unsampled* input x, so the convs run on the [160, 64] grid instead of
[320, 128] (43% fewer MACs, no materialized upsample).

All tensors on chip live in a "parity layout": partition p = ch + 32*(2a+c),
free dim = (i, j) on the [160, 64] parity grid. conv01 (the second hex conv,
on the full-resolution h) becomes, per output parity, a sum of 4 matmuls
whose contraction dim K = 32 channels x 4 input parity groups = 128 - the
parity groups are already stacked in partitions, so no data duplication.

Matmuls: K=128, M=32, N=512, 4-way column tiling via tile_position so the
four parity groups of one output tile run concurrently in the PE array.
conv00/conv10 use float32r (TF32-like, full rate, no cast pass from the f32
input); conv01 runs in bf16 (h is produced in bf16 by the scalar engine).
"""

import sys

for _p in ("/opt/trn_rl_repo", "/root/.axon_site/_ro/trn_rl_repo"):
    if _p not in sys.path:
        sys.path.insert(0, _p)

import numpy as np
import ml_dtypes

import concourse.bass as bass
import concourse.bacc as bacc
import concourse.mybir as mybir
import concourse.tile as tile
from concourse import bass_utils

F32 = mybir.dt.float32
F32R = mybir.dt.float32r
BF16 = mybir.dt.bfloat16
ALU = mybir.AluOpType
ACT = mybir.ActivationFunctionType
AX = mybir.AxisListType

N_CORES = 8
B, CIN, COUT = 16, 64, 32
HX, WX = 160, 64          # x spatial (per sample); parity grid is the same size
B_LOC = B // N_CORES      # 2 samples per core
NCH = HX // 8             # 20 chunks of 8 parity-grid rows per sample
NTILE = B_LOC * NCH       # 40 [128, 512] tiles per intermediate tensor
CPX = 512                 # elements per chunk per partition (8 rows x 64 cols)
HW2 = (HX + 2) * (WX + 2)  # 162*66 halo-padded plane, per partition
BN_EPS = 1e-5
N_STAT = float(B * 2 * HX * 2 * WX)  # BN stat count per channel: 16*320*128
NSLOT = 96

_HEX = np.ones((3, 3), np.float32)
_HEX[0, 2] = 0.0
_HEX[2, 0] = 0.0

_A = {0: np.array([[1, 0, 0], [0, 1, 1]], np.float32),
      1: np.array([[1, 1, 0], [0, 0, 1]], np.float32)}


def _fold_up(w):
    """w [Cout, Cin, 3, 3] -> lhsT blocks [8, 128, 32], idx = a*4 + c*2 + u.

    Block rows: ci + 64*v (v = column-shift tap), cols: co.
    """
    wt = w * _HEX
    out = np.zeros((8, 128, 32), np.float32)
    for a in range(2):
        for c in range(2):
            e = np.einsum('ud,ve,oide->uvoi', _A[a], _A[c], wt)  # [2,2,Cout,Cin]
            for u in range(2):
                blk = np.zeros((128, 32), np.float32)
                for v in range(2):
                    blk[64 * v:64 * v + 64, :] = e[u, v].T
                out[a * 4 + c * 2 + u] = blk
    return out


def _fold_c01(w):
    """w01 [Cout, 32, 3, 3] -> lhsT blocks [16, 128, 32], idx = al*8+ga*4+d*2+e.

    Block rows: ci + 32*(2*a' + c'), cols: co.
    """
    wt = w * _HEX
    wd = np.zeros((2, 2, 2, 2, 2, 2, w.shape[0], w.shape[1]), np.float32)
    for al in range(2):
        for dy in range(3):
            ap_ = (al + dy - 1) % 2
            d = (al + dy - 1) // 2 - (al - 1)
            for ga in range(2):
                for dx in range(3):
                    cp_ = (ga + dx - 1) % 2
                    e = (ga + dx - 1) // 2 - (ga - 1)
                    wd[al, ga, d, e, ap_, cp_] += wt[:, :, dy, dx]
    out = np.zeros((16, 128, 32), np.float32)
    for al in range(2):
        for ga in range(2):
            for d in range(2):
                for e in range(2):
                    blk = np.zeros((128, 32), np.float32)
                    for ap_ in range(2):
                        for cp_ in range(2):
                            g = 2 * ap_ + cp_
                            blk[32 * g:32 * g + 32, :] = wd[al, ga, d, e, ap_, cp_].T
                    out[al * 8 + ga * 4 + d * 2 + e] = blk
    return out


def _emit(nc, tc, x_in, w00l, w10l, w01l, gb, out):
    with (
        tc.tile_pool(name="setup", bufs=1) as setup,
        tc.tile_pool(name="dram", bufs=1, space="DRAM") as dram,
        tc.tile_pool(name="stores", bufs=1) as stores,
        tc.tile_pool(name="scrap", bufs=3) as scrap,
        tc.tile_pool(name="ps", bufs=6, space="PSUM") as ps,
    ):
        # --- weights / gamma-beta ---
        wu00 = setup.tile([128, 8 * 32], BF16)
        wu10 = setup.tile([128, 8 * 32], BF16)
        w01t = setup.tile([128, 16 * 32], BF16)
        nc.sync.dma_start(wu00[:].rearrange("p (b c) -> p b c", b=8),
                          w00l.rearrange("b p c -> p b c"))
        nc.sync.dma_start(wu10[:].rearrange("p (b c) -> p b c", b=8),
                          w10l.rearrange("b p c -> p b c"))
        nc.sync.dma_start(w01t[:].rearrange("p (b c) -> p b c", b=16),
                          w01l.rearrange("b p c -> p b c"))
        gbt = setup.tile([32, 6], F32)  # (g00,b00,g01,b01,g10,b10) per channel
        nc.sync.dma_start(gbt[:], gb.rearrange("t c -> c t"))

        # --- stats buffers ---
        ssum = [setup.tile([128, NSLOT], F32, tag=f"ssum{t}", name=f"ssum{t}") for t in range(3)]
        ssq = [setup.tile([128, NSLOT], F32, tag=f"ssq{t}", name=f"ssq{t}") for t in range(3)]
        pp = [setup.tile([128, 2], F32, tag=f"pp{t}", name=f"pp{t}") for t in range(3)]
        dpp = [dram.tile([128, 2], F32, tag=f"dpp{t}", name=f"dpp{t}") for t in range(3)]
        gath = [setup.tile([32, 8], F32, tag=f"gath{t}", name=f"gath{t}") for t in range(3)]
        loc = [setup.tile([32, 2], F32, tag=f"loc{t}", name=f"loc{t}") for t in range(3)]

        ar1_in = dram.tile([32, 2], F32, tag="ar1i")
        ar1_out = dram.tile([32, 2], F32, tag="ar1o")
        ar2_in = dram.tile([32, 4], F32, tag="ar2i")
        ar2_out = dram.tile([32, 4], F32, tag="ar2o")

        sg1 = setup.tile([32, 2], F32)   # AR1 result (c00: sum, sumsq)
        sg2 = setup.tile([32, 4], F32)   # AR2 result (c10 | c01)

        st0 = setup.tile([32, 2], F32)   # (scale0, shift0) for BN0
        st0r = setup.tile([128, 2], F32)
        fin = setup.tile([32, 3], F32)   # (r = s2/s1, s1, b1+b2)
        finr = setup.tile([128, 3], F32)
        sm = setup.tile([32, 12], F32)   # small-math scratch
        eps = setup.tile([32, 1], F32)
        nc.gpsimd.memset(eps[:], BN_EPS)

        # --- intermediate stores (bf16, parity layout) ---
        c00s = stores.tile([128, NTILE * CPX], BF16, tag="c00")
        c10s = stores.tile([128, NTILE * CPX], BF16, tag="c10")

        def _copy_evac(dst, src_ap, t, slot):
            """PSUM -> bf16 store copy with free per-partition sum accumulation."""
            if slot % 3 == 2:  # 1/3 of evacuations on the vector engine
                nc.vector.tensor_scalar(dst, src_ap, 1.0, 0.0, ALU.mult,
                                        ALU.add,
                                        accum_out=ssum[t][:, slot:slot + 1])
            else:
                nc.scalar.activation(dst, src_ap, ACT.Copy,
                                     accum_out=ssum[t][:, slot:slot + 1])

        def _sumsq(store, idx, t, slot):
            dst = store[:, idx * CPX:(idx + 1) * CPX]
            sq = scrap.tile([128, CPX], BF16, tag="sqscrap")
            nc.vector.scalar_tensor_tensor(
                sq[:], dst, 1.0, dst, ALU.mult, ALU.mult,
                accum_out=ssq[t][:, slot:slot + 1])

        def _evac(pt, store, t, idx):
            """Parity-layout evacuation (c00): one [128, 512] copy."""
            dst = store[:, idx * CPX:(idx + 1) * CPX]
            _copy_evac(dst, pt[:], t, idx)
            _sumsq(store, idx, t, idx)

        def _evac2(pt0, pt1, store, t, idx):
            """W-interleaving evacuation (c10/c01): two [128, 256] strided copies.

            pt0/pt1 hold the c=0 / c=1 W-parity halves; partition groups are
            (a, row-half); free layout becomes (i 4, W 128) with W = 2j + c.
            """
            dstv = store[:, idx * CPX:(idx + 1) * CPX].rearrange(
                "p (i j c) -> p i j c", i=4, c=2)
            _copy_evac(dstv[:, :, :, 0], pt0[:], t, 2 * idx)
            _copy_evac(dstv[:, :, :, 1], pt1[:], t, 2 * idx + 1)
            _sumsq(store, idx, t, idx)

        def stats_combine(t, dst_ap, nsum=NTILE, nsq=NTILE):
            nc.vector.reduce_sum(pp[t][:, 0:1], ssum[t][:, 0:nsum], axis=AX.X)
            nc.vector.reduce_sum(pp[t][:, 1:2], ssq[t][:, 0:nsq], axis=AX.X)
            nc.sync.dma_start(dpp[t][:], pp[t][:])
            nc.sync.dma_start(gath[t][:].rearrange("p (g i) -> p g i", g=4),
                              dpp[t][:].rearrange("(g p) i -> p g i", p=32))
            g = gath[t]
            nc.vector.tensor_add(g[:, 0:2], g[:, 0:2], g[:, 2:4])
            nc.vector.tensor_add(g[:, 4:6], g[:, 4:6], g[:, 6:8])
            nc.vector.tensor_add(loc[t][:], g[:, 0:2], g[:, 4:6])
            nc.sync.dma_start(dst_ap, loc[t][:])

        def bn_coeffs(stats2, gamma_ap, beta_ap, s_dst, t_dst):
            """stats2 [32,2]=(sum,sumsq) -> BN scale/shift into s_dst/t_dst."""
            mv = sm[:, 0:2]
            nc.vector.tensor_scalar(mv, stats2, 1.0 / N_STAT, None, ALU.mult)
            m2 = sm[:, 2:3]
            nc.vector.tensor_mul(m2, mv[:, 0:1], mv[:, 0:1])
            var = sm[:, 3:4]
            nc.vector.tensor_sub(var, mv[:, 1:2], m2)
            std = sm[:, 2:3]  # reuse
            nc.scalar.activation(std, var, ACT.Sqrt, bias=eps[:])
            rinv = sm[:, 3:4]  # reuse
            nc.vector.reciprocal(rinv, std)
            nc.vector.tensor_mul(s_dst, rinv, gamma_ap)
            msc = sm[:, 2:3]
            nc.vector.tensor_mul(msc, mv[:, 0:1], s_dst)
            nc.vector.tensor_sub(t_dst, beta_ap, msc)

        # ================= phase A: conv00 / conv10 ==========================
        with tc.tile_pool(name="x2", bufs=2) as x2p:
            x2 = []
            for s in range(B_LOC):
                t = x2p.tile([128, HW2], BF16, tag="x2")
                v = t[:].rearrange("p (r c) -> p r c", c=WX + 2)
                # zero borders: orig half cols {0, 65}; dup half cols {64, 65}
                nc.gpsimd.memset(v[0:64, :, 0:1], 0.0)
                nc.gpsimd.memset(v[0:64, :, WX + 1:WX + 2], 0.0)
                nc.gpsimd.memset(v[64:128, :, WX:WX + 2], 0.0)
                rb = HX // 4
                for q in range(4):
                    src = x_in[s, :, q * rb:(q + 1) * rb, :]
                    nc.sync.dma_start(
                        v[0:64, 1 + q * rb:1 + (q + 1) * rb, 1:WX + 1], src)
                    nc.sync.dma_start(
                        v[64:128, 1 + q * rb:1 + (q + 1) * rb, 0:WX], src)
                # wrap halo rows
                nc.sync.dma_start(v[0:64, 0:1, 1:WX + 1], x_in[s, :, HX - 1:HX, :])
                nc.sync.dma_start(v[0:64, HX + 1:HX + 2, 1:WX + 1], x_in[s, :, 0:1, :])
                nc.sync.dma_start(v[64:128, 0:1, 0:WX], x_in[s, :, HX - 1:HX, :])
                nc.sync.dma_start(v[64:128, HX + 1:HX + 2, 0:WX], x_in[s, :, 0:1, :])
                x2.append(v)

            def upconv_parity(wt, store, tensor_idx):
                """Parity-layout up-conv (c00): groups (a, c), N=512."""
                for s in range(B_LOC):
                    for k in range(NCH):
                        idx = s * NCH + k
                        pt = ps.tile([128, CPX], F32, tag="ps", bufs=3)
                        for g in range(4):
                            a, c = g >> 1, g & 1
                            for u in range(2):
                                widx = a * 4 + c * 2 + u
                                rhs = x2[s][:, 8 * k + u + a:8 * k + u + a + 8,
                                            c:c + WX]
                                nc.tensor.matmul(
                                    pt[32 * g:32 * g + 32, :],
                                    wt[:, widx * 32:(widx + 1) * 32],
                                    rhs, start=(u == 0), stop=(u == 1),
                                    tile_position=(0, 32 * g))
                        _evac(pt, store, tensor_idx, idx)

            def upconv_il(wt, store, tensor_idx):
                """W-interleaved up-conv (c10): groups (a, row-half), N=256."""
                for s in range(B_LOC):
                    for k in range(NCH):
                        idx = s * NCH + k
                        pts = [ps.tile([128, CPX // 2], F32, tag=f"psh{c}",
                                       bufs=2, name=f"psh{c}")
                               for c in range(2)]
                        for c in range(2):
                            for g in range(4):
                                a, ih = g >> 1, g & 1
                                r0 = 8 * k + 4 * ih
                                for u in range(2):
                                    widx = a * 4 + c * 2 + u
                                    rhs = x2[s][:, r0 + u + a:r0 + u + a + 4,
                                                c:c + WX]
                                    nc.tensor.matmul(
                                        pts[c][32 * g:32 * g + 32, :],
                                        wt[:, widx * 32:(widx + 1) * 32],
                                        rhs, start=(u == 0), stop=(u == 1),
                                        tile_position=(0, 32 * g))
                        _evac2(pts[0], pts[1], store, tensor_idx, idx)

            upconv_parity(wu00, c00s, 0)
            stats_combine(0, ar1_in[:])
            nc.gpsimd.collective_compute(
                "AllReduce", ALU.add, replica_groups=[list(range(N_CORES))],
                ins=[ar1_in.opt()], outs=[ar1_out.opt()])
            nc.sync.dma_start(sg1[:], ar1_out[:])
            bn_coeffs(sg1[:], gbt[:, 0:1], gbt[:, 1:2], st0[:, 0:1], st0[:, 1:2])
            for g in range(4):
                nc.sync.dma_start(st0r[32 * g:32 * g + 32, :], st0[:])

            upconv_il(wu10, c10s, 1)  # overlaps AllReduce #1
            stats_combine(1, ar2_in[:, 0:2], nsum=2 * NTILE, nsq=NTILE)

        # ================= phase B: h = relu(BN0(c00)); conv01 ===============
        with (
            tc.tile_pool(name="h", bufs=2) as hp,
            tc.tile_pool(name="c01p", bufs=1) as c01p,
        ):
            c01s = c01p.tile([128, NTILE * CPX], BF16, tag="c01")
            for s in range(B_LOC):
                ht = hp.tile([128, HW2], BF16, tag="h")
                hv = ht[:].rearrange("p (r c) -> p r c", c=WX + 2)
                nc.gpsimd.memset(hv[:, :, 0:1], 0.0)
                nc.gpsimd.memset(hv[:, :, WX + 1:WX + 2], 0.0)
                for k in [NCH - 1] + list(range(NCH - 1)):
                    idx = s * NCH + k
                    src = c00s[:, idx * CPX:(idx + 1) * CPX].rearrange(
                        "p (r c) -> p r c", c=WX)
                    nc.scalar.activation(hv[:, 8 * k + 1:8 * k + 9, 1:WX + 1],
                                         src, ACT.Relu, scale=st0r[:, 0:1],
                                         bias=st0r[:, 1:2])
                # wrap halo rows (within each parity group)
                nc.vector.tensor_copy(hv[:, 0:1, :], hv[:, HX:HX + 1, :])
                nc.vector.tensor_copy(hv[:, HX + 1:HX + 2, :], hv[:, 1:2, :])

                for k in range(NCH):
                    idx = s * NCH + k
                    pts = [ps.tile([128, CPX // 2], F32, tag=f"psh{c}",
                                   bufs=2, name=f"psh{c}")
                           for c in range(2)]
                    for ga in range(2):
                        for m in range(4):
                            al, ih = m >> 1, m & 1
                            r0 = 8 * k + 4 * ih
                            first = True
                            for d in range(2):
                                for e in range(2):
                                    widx = al * 8 + ga * 4 + d * 2 + e
                                    rhs = hv[:, r0 + d + al:r0 + d + al + 4,
                                             e + ga:e + ga + WX]
                                    nc.tensor.matmul(
                                        pts[ga][32 * m:32 * m + 32, :],
                                        w01t[:, widx * 32:(widx + 1) * 32],
                                        rhs, start=first,
                                        stop=(d == 1 and e == 1),
                                        tile_position=(0, 32 * m))
                                    first = False
                    _evac2(pts[0], pts[1], c01s, 2, idx)

            stats_combine(2, ar2_in[:, 2:4], nsum=2 * NTILE, nsq=NTILE)
            nc.gpsimd.collective_compute(
                "AllReduce", ALU.add, replica_groups=[list(range(N_CORES))],
                ins=[ar2_in.opt()], outs=[ar2_out.opt()])
            nc.sync.dma_start(sg2[:], ar2_out[:])

            # BN1 (c01, g01/b01) and BN2 (c10, g10/b10)
            s1, t1 = sm[:, 4:5], sm[:, 5:6]
            s2, t2 = sm[:, 6:7], sm[:, 7:8]
            bn_coeffs(sg2[:, 2:4], gbt[:, 2:3], gbt[:, 3:4], s1, t1)
            bn_coeffs(sg2[:, 0:2], gbt[:, 4:5], gbt[:, 5:6], s2, t2)
            rs1 = sm[:, 8:9]
            nc.vector.reciprocal(rs1, s1)
            nc.vector.tensor_mul(fin[:, 0:1], s2, rs1)   # r = s2/s1
            nc.vector.tensor_copy(fin[:, 1:2], s1)
            nc.vector.tensor_add(fin[:, 2:3], t1, t2)    # b' = t1 + t2
            for g in range(4):
                nc.sync.dma_start(finr[32 * g:32 * g + 32, :], fin[:])

            # ============ phase C: out = relu(s1*(c01 + r*c10) + b') =========
            # stores are W-interleaved: partition ch + 32*(2a + ih),
            # free (i 4, W 128); out row H = 2*(8k + 4*ih + i) + a.
            ov = out.rearrange("s ch (i2 a) w -> s a ch i2 w", a=2)
            for s in range(B_LOC):
                for k in range(NCH):
                    idx = s * NCH + k
                    tmp = scrap.tile([128, CPX], BF16, tag="fintmp")
                    nc.vector.scalar_tensor_tensor(
                        tmp[:], c10s[:, idx * CPX:(idx + 1) * CPX],
                        finr[:, 0:1], c01s[:, idx * CPX:(idx + 1) * CPX],
                        ALU.mult, ALU.add)
                    ot = scrap.tile([128, CPX], F32, tag="finout")
                    nc.scalar.activation(ot[:], tmp[:], ACT.Relu,
                                         scale=finr[:, 1:2], bias=finr[:, 2:3])
                    for g in range(4):
                        a, ih = g >> 1, g & 1
                        dst = ov[s, a][:, 8 * k + 4 * ih:8 * k + 4 * ih + 4, :]
                        src = ot[32 * g:32 * g + 32, :].rearrange(
                            "ch (i w) -> ch i w", w=2 * WX)
                        nc.sync.dma_start(dst, src)


def _build_nc(repeat=1):
    nc = bacc.Bacc("TRN2", target_bir_lowering=False, debug=False,
                   num_devices=N_CORES)
    x_in = nc.dram_tensor("x", [B_LOC, CIN, HX, WX], BF16,
                          kind="ExternalInput").ap()
    w00l = nc.dram_tensor("w00l", [8, 128, 32], BF16, kind="ExternalInput").ap()
    w10l = nc.dram_tensor("w10l", [8, 128, 32], BF16, kind="ExternalInput").ap()
    w01l = nc.dram_tensor("w01l", [16, 128, 32], BF16, kind="ExternalInput").ap()
    gb = nc.dram_tensor("gb", [6, 32], F32, kind="ExternalInput").ap()
    out = nc.dram_tensor("out", [B_LOC, COUT, 2 * HX, 2 * WX], F32,
                         kind="ExternalOutput").ap()
    with tile.TileContext(nc) as tc:
        for _ in range(repeat):
            _emit(nc, tc, x_in, w00l, w10l, w01l, gb, out)
    nc.compile()
    return nc


class _Runner:
    """Persistent jitted SPMD executor (mirrors bass2jax.run_bass_via_pjrt,
    but builds the jit once so steady-state calls skip tracing/compile)."""

    def __init__(self, nc):
        import jax
        from jax.sharding import Mesh, PartitionSpec
        from jax.experimental.shard_map import shard_map
        from concourse import bass2jax

        bass2jax.install_neuronx_cc_hook()
        self._jax = jax
        in_names, out_names, out_avals, zero_templates = [], [], [], []
        partition_name = (nc.partition_id_tensor.name
                          if nc.partition_id_tensor else None)
        for alloc in nc.m.functions[0].allocations:
            if not isinstance(alloc, mybir.MemoryLocationSet):
                continue
            name = alloc.memorylocations[0].name
            if alloc.kind == "ExternalInput":
                if name != partition_name:
                    in_names.append(name)
            elif alloc.kind == "ExternalOutput":
                out_names.append(name)
                shape = tuple(alloc.tensor_shape)
                dtype = mybir.dt.np(alloc.dtype)
                out_avals.append(jax.core.ShapedArray(shape, dtype))
                zero_templates.append((shape, dtype))
        self.in_names, self.out_names = in_names, out_names
        self.zero_templates = zero_templates
        self.out_avals = out_avals
        n_params, n_outs = len(in_names), len(out_names)
        all_names = list(in_names) + list(out_names)
        if partition_name is not None:
            all_names.append(partition_name)

        def _body(*args):
            operands = list(args)
            if partition_name is not None:
                operands.append(bass2jax.partition_id_tensor())
            outs = bass2jax._bass_exec_p.bind(
                *operands,
                out_avals=tuple(out_avals),
                in_names=tuple(all_names),
                out_names=tuple(out_names),
                lowering_input_output_aliases=(),
                sim_require_finite=True,
                sim_require_nnan=True,
                nc=nc,
            )
            return tuple(outs)

        devices = jax.devices()[:N_CORES]
        mesh = Mesh(np.asarray(devices), ("core",))
        donate = tuple(range(n_params, n_params + n_outs))
        self._fn = jax.jit(
            shard_map(_body, mesh=mesh,
                      in_specs=(PartitionSpec("core"),) * (n_params + n_outs),
                      out_specs=(PartitionSpec("core"),) * n_outs,
                      check_rep=False),
            donate_argnums=donate, keep_unused=True)

    def __call__(self, in_maps):
        concat_in = [np.concatenate([np.asarray(m[name]) for m in in_maps],
                                    axis=0) for name in self.in_names]
        zeros = [np.zeros((N_CORES * s[0],) + tuple(s[1:]), d)
                 for (s, d) in self.zero_templates]
        outs = self._fn(*concat_in, *zeros)
        return [
            {name: np.asarray(outs[i]).reshape(N_CORES, *self.out_avals[i].shape)[c]
             for i, name in enumerate(self.out_names)}
            for c in range(N_CORES)
        ]


_CACHE = {}


def _get_nc():
    if "nc" not in _CACHE:
        _CACHE["nc"] = _build_nc()
    return _CACHE["nc"]


def _make_in_maps(inputs):
    x = np.ascontiguousarray(
        np.asarray(inputs["x"], dtype=np.float32).astype(ml_dtypes.bfloat16))
    gb = np.stack([np.asarray(inputs[k], dtype=np.float32)
                   for k in ("g00", "b00", "g01", "b01", "g10", "b10")])
    w00l = _fold_up(np.asarray(inputs["w00"], dtype=np.float32)).astype(
        ml_dtypes.bfloat16)
    w10l = _fold_up(np.asarray(inputs["w10"], dtype=np.float32)).astype(
        ml_dtypes.bfloat16)
    w01l = _fold_c01(np.asarray(inputs["w01"], dtype=np.float32)).astype(
        ml_dtypes.bfloat16)
    return [{"x": x[i * B_LOC:(i + 1) * B_LOC],
             "w00l": w00l, "w10l": w10l, "w01l": w01l, "gb": gb}
            for i in range(N_CORES)]


def kernel(**inputs) -> np.ndarray:
    in_maps = _make_in_maps(inputs)
    if "runner" not in _CACHE:
        nc = _get_nc()
        # First call goes through the standard entry point (compiles the NEFF
        # and executes on cores 0-7); later calls reuse a cached jit so they
        # skip host-side retracing.
        res = bass_utils.run_bass_kernel_spmd(nc, in_maps,
                                              core_ids=list(range(N_CORES)))
        _CACHE["runner"] = _Runner(nc)
        return np.concatenate([r["out"] for r in res.results], axis=0)
    results = _CACHE["runner"](in_maps)
    return np.concatenate([r["out"] for r in results], axis=0)


# revision 23
# speedup vs baseline: 85479.5053x; 5.9506x over previous
"""Trainium2 Bass kernel for BasicIcoS2SUpBlock (upsample + hex-conv ×3 + sync-BN ×3).

Strategy
--------
Data parallel over batch: B=16 -> 2 samples per core on 8 NeuronCores.
Sync-BN via two tiny AllReduce collectives of per-channel (sum, sumsq).

The nearest-neighbor 2x upsample is folded into conv00/conv10: each output
parity class (a, c) in {0,1}^2 sees an effective 2x2 kernel on the
*unsampled* input x, so the convs run on the [160, 64] grid instead of
[320, 128] (43% fewer MACs, no materialized upsample).

All tensors on chip live in a "parity layout": partition p = ch + 32*(2a+c),
free dim = (i, j) on the [160, 64] parity grid. conv01 (the second hex conv,
on the full-resolution h) becomes, per output parity, a sum of 4 matmuls
whose contraction dim K = 32 channels x 4 input parity groups = 128 - the
parity groups are already stacked in partitions, so no data duplication.

Matmuls: K=128, M=32, N=512, 4-way column tiling via tile_position so the
four parity groups of one output tile run concurrently in the PE array.
conv00/conv10 use float32r (TF32-like, full rate, no cast pass from the f32
input); conv01 runs in bf16 (h is produced in bf16 by the scalar engine).
"""

import sys

for _p in ("/opt/trn_rl_repo", "/root/.axon_site/_ro/trn_rl_repo"):
    if _p not in sys.path:
        sys.path.insert(0, _p)

import numpy as np
import ml_dtypes

import concourse.bass as bass
import concourse.bacc as bacc
import concourse.mybir as mybir
import concourse.tile as tile
from concourse import bass_utils

F32 = mybir.dt.float32
F32R = mybir.dt.float32r
BF16 = mybir.dt.bfloat16
ALU = mybir.AluOpType
ACT = mybir.ActivationFunctionType
AX = mybir.AxisListType

N_CORES = 8
B, CIN, COUT = 16, 64, 32
HX, WX = 160, 64          # x spatial (per sample); parity grid is the same size
B_LOC = B // N_CORES      # 2 samples per core
NCH = HX // 8             # 20 chunks of 8 parity-grid rows per sample
NTILE = B_LOC * NCH       # 40 [128, 512] tiles per intermediate tensor
CPX = 512                 # elements per chunk per partition (8 rows x 64 cols)
HW2 = (HX + 2) * (WX + 2)  # 162*66 halo-padded plane, per partition
BN_EPS = 1e-5
N_STAT = float(B * 2 * HX * 2 * WX)  # BN stat count per channel: 16*320*128
NSLOT = 96

_HEX = np.ones((3, 3), np.float32)
_HEX[0, 2] = 0.0
_HEX[2, 0] = 0.0

_A = {0: np.array([[1, 0, 0], [0, 1, 1]], np.float32),
      1: np.array([[1, 1, 0], [0, 0, 1]], np.float32)}


def _fold_up(w):
    """w [Cout, Cin, 3, 3] -> lhsT blocks [8, 128, 32], idx = a*4 + c*2 + u.

    Block rows: ci + 64*v (v = column-shift tap), cols: co.
    """
    wt = w * _HEX
    out = np.zeros((8, 128, 32), np.float32)
    for a in range(2):
        for c in range(2):
            e = np.einsum('ud,ve,oide->uvoi', _A[a], _A[c], wt)  # [2,2,Cout,Cin]
            for u in range(2):
                blk = np.zeros((128, 32), np.float32)
                for v in range(2):
                    blk[64 * v:64 * v + 64, :] = e[u, v].T
                out[a * 4 + c * 2 + u] = blk
    return out


def _fold_c01(w):
    """w01 [Cout, 32, 3, 3] -> lhsT blocks [16, 128, 32], idx = al*8+ga*4+d*2+e.

    Block rows: ci + 32*(2*a' + c'), cols: co.
    """
    wt = w * _HEX
    wd = np.zeros((2, 2, 2, 2, 2, 2, w.shape[0], w.shape[1]), np.float32)
    for al in range(2):
        for dy in range(3):
            ap_ = (al + dy - 1) % 2
            d = (al + dy - 1) // 2 - (al - 1)
            for ga in range(2):
                for dx in range(3):
                    cp_ = (ga + dx - 1) % 2
                    e = (ga + dx - 1) // 2 - (ga - 1)
                    wd[al, ga, d, e, ap_, cp_] += wt[:, :, dy, dx]
    out = np.zeros((16, 128, 32), np.float32)
    for al in range(2):
        for ga in range(2):
            for d in range(2):
                for e in range(2):
                    blk = np.zeros((128, 32), np.float32)
                    for ap_ in range(2):
                        for cp_ in range(2):
                            g = 2 * ap_ + cp_
                            blk[32 * g:32 * g + 32, :] = wd[al, ga, d, e, ap_, cp_].T
                    out[al * 8 + ga * 4 + d * 2 + e] = blk
    return out


def _emit(nc, tc, x_in, w00l, w10l, w01l, gb, out):
    with (
        tc.tile_pool(name="setup", bufs=1) as setup,
        tc.tile_pool(name="dram", bufs=1, space="DRAM") as dram,
        tc.tile_pool(name="stores", bufs=1) as stores,
        tc.tile_pool(name="scrap", bufs=3) as scrap,
        tc.tile_pool(name="ps", bufs=6, space="PSUM") as ps,
    ):
        # --- weights / gamma-beta ---
        wu00 = setup.tile([128, 8 * 32], BF16)
        wu10 = setup.tile([128, 8 * 32], BF16)
        w01t = setup.tile([128, 16 * 32], BF16)
        nc.sync.dma_start(wu00[:].rearrange("p (b c) -> p b c", b=8),
                          w00l.rearrange("b p c -> p b c"))
        nc.sync.dma_start(wu10[:].rearrange("p (b c) -> p b c", b=8),
                          w10l.rearrange("b p c -> p b c"))
        nc.sync.dma_start(w01t[:].rearrange("p (b c) -> p b c", b=16),
                          w01l.rearrange("b p c -> p b c"))
        gbt = setup.tile([32, 6], F32)  # (g00,b00,g01,b01,g10,b10) per channel
        nc.sync.dma_start(gbt[:], gb.rearrange("t c -> c t"))

        # --- stats buffers ---
        ssum = [setup.tile([128, NSLOT], F32, tag=f"ssum{t}", name=f"ssum{t}") for t in range(3)]
        ssq = [setup.tile([128, NSLOT], F32, tag=f"ssq{t}", name=f"ssq{t}") for t in range(3)]
        pp = [setup.tile([128, 2], F32, tag=f"pp{t}", name=f"pp{t}") for t in range(3)]
        dpp = [dram.tile([128, 2], F32, tag=f"dpp{t}", name=f"dpp{t}") for t in range(3)]
        gath = [setup.tile([32, 8], F32, tag=f"gath{t}", name=f"gath{t}") for t in range(3)]
        loc = [setup.tile([32, 2], F32, tag=f"loc{t}", name=f"loc{t}") for t in range(3)]

        ar1_in = dram.tile([32, 2], F32, tag="ar1i")
        ar1_out = dram.tile([32, 2], F32, tag="ar1o")
        ar2_in = dram.tile([32, 4], F32, tag="ar2i")
        ar2_out = dram.tile([32, 4], F32, tag="ar2o")

        sg1 = setup.tile([32, 2], F32)   # AR1 result (c00: sum, sumsq)
        sg2 = setup.tile([32, 4], F32)   # AR2 result (c10 | c01)

        st0 = setup.tile([32, 2], F32)   # (scale0, shift0) for BN0
        st0r = setup.tile([128, 2], F32)
        fin = setup.tile([32, 3], F32)   # (r = s2/s1, s1, b1+b2)
        finr = setup.tile([128, 3], F32)
        sm = setup.tile([32, 12], F32)   # small-math scratch
        eps = setup.tile([32, 1], F32)
        nc.gpsimd.memset(eps[:], BN_EPS)

        # --- intermediate stores (bf16, parity layout) ---
        c00s = stores.tile([128, NTILE * CPX], BF16, tag="c00")
        c10s = stores.tile([128, NTILE * CPX], BF16, tag="c10")

        def _copy_evac(dst, src_ap, t, slot):
            """PSUM -> bf16 store copy with free per-partition sum accumulation."""
            if slot % 3 == 2:  # 1/3 of evacuations on the vector engine
                nc.vector.tensor_scalar(dst, src_ap, 1.0, 0.0, ALU.mult,
                                        ALU.add,
                                        accum_out=ssum[t][:, slot:slot + 1])
            else:
                nc.scalar.activation(dst, src_ap, ACT.Copy,
                                     accum_out=ssum[t][:, slot:slot + 1])

        def _sumsq(store, idx, t, slot):
            dst = store[:, idx * CPX:(idx + 1) * CPX]
            sq = scrap.tile([128, CPX], BF16, tag="sqscrap")
            nc.vector.scalar_tensor_tensor(
                sq[:], dst, 1.0, dst, ALU.mult, ALU.mult,
                accum_out=ssq[t][:, slot:slot + 1])

        def _evac(pt, store, t, idx):
            """Parity-layout evacuation (c00): one [128, 512] copy."""
            dst = store[:, idx * CPX:(idx + 1) * CPX]
            _copy_evac(dst, pt[:], t, idx)
            _sumsq(store, idx, t, idx)

        def _evac2(pt0, pt1, store, t, idx):
            """W-interleaving evacuation (c10/c01): two [128, 256] strided copies.

            pt0/pt1 hold the c=0 / c=1 W-parity halves; partition groups are
            (a, row-half); free layout becomes (i 4, W 128) with W = 2j + c.
            """
            dstv = store[:, idx * CPX:(idx + 1) * CPX].rearrange(
                "p (i j c) -> p i j c", i=4, c=2)
            _copy_evac(dstv[:, :, :, 0], pt0[:], t, 2 * idx)
            _copy_evac(dstv[:, :, :, 1], pt1[:], t, 2 * idx + 1)
            _sumsq(store, idx, t, idx)

        def stats_combine(t, dst_ap, nsum=NTILE, nsq=NTILE):
            nc.vector.reduce_sum(pp[t][:, 0:1], ssum[t][:, 0:nsum], axis=AX.X)
            nc.vector.reduce_sum(pp[t][:, 1:2], ssq[t][:, 0:nsq], axis=AX.X)
            nc.sync.dma_start(dpp[t][:], pp[t][:])
            nc.sync.dma_start(gath[t][:].rearrange("p (g i) -> p g i", g=4),
                              dpp[t][:].rearrange("(g p) i -> p g i", p=32))
            g = gath[t]
            nc.vector.tensor_add(g[:, 0:2], g[:, 0:2], g[:, 2:4])
            nc.vector.tensor_add(g[:, 4:6], g[:, 4:6], g[:, 6:8])
            nc.vector.tensor_add(loc[t][:], g[:, 0:2], g[:, 4:6])
            nc.sync.dma_start(dst_ap, loc[t][:])

        def bn_coeffs(stats2, gamma_ap, beta_ap, s_dst, t_dst):
            """stats2 [32,2]=(sum,sumsq) -> BN scale/shift into s_dst/t_dst."""
            mv = sm[:, 0:2]
            nc.vector.tensor_scalar(mv, stats2, 1.0 / N_STAT, None, ALU.mult)
            m2 = sm[:, 2:3]
            nc.vector.tensor_mul(m2, mv[:, 0:1], mv[:, 0:1])
            var = sm[:, 3:4]
            nc.vector.tensor_sub(var, mv[:, 1:2], m2)
            std = sm[:, 2:3]  # reuse
            nc.scalar.activation(std, var, ACT.Sqrt, bias=eps[:])
            rinv = sm[:, 3:4]  # reuse
            nc.vector.reciprocal(rinv, std)
            nc.vector.tensor_mul(s_dst, rinv, gamma_ap)
            msc = sm[:, 2:3]
            nc.vector.tensor_mul(msc, mv[:, 0:1], s_dst)
            nc.vector.tensor_sub(t_dst, beta_ap, msc)

        # ================= phase A: conv00 / conv10 ==========================
        with tc.tile_pool(name="x2", bufs=2) as x2p:
            x2 = []
            for s in range(B_LOC):
                t = x2p.tile([128, HW2], BF16, tag="x2")
                v = t[:].rearrange("p (r c) -> p r c", c=WX + 2)
                # zero borders: orig half cols {0, 65}; dup half cols {64, 65}
                nc.gpsimd.memset(v[0:64, :, 0:1], 0.0)
                nc.gpsimd.memset(v[0:64, :, WX + 1:WX + 2], 0.0)
                nc.gpsimd.memset(v[64:128, :, WX:WX + 2], 0.0)
                rb = HX // 4
                for q in range(4):
                    src = x_in[s, :, q * rb:(q + 1) * rb, :]
                    nc.sync.dma_start(
                        v[0:64, 1 + q * rb:1 + (q + 1) * rb, 1:WX + 1], src)
                    nc.sync.dma_start(
                        v[64:128, 1 + q * rb:1 + (q + 1) * rb, 0:WX], src)
                # wrap halo rows
                nc.sync.dma_start(v[0:64, 0:1, 1:WX + 1], x_in[s, :, HX - 1:HX, :])
                nc.sync.dma_start(v[0:64, HX + 1:HX + 2, 1:WX + 1], x_in[s, :, 0:1, :])
                nc.sync.dma_start(v[64:128, 0:1, 0:WX], x_in[s, :, HX - 1:HX, :])
                nc.sync.dma_start(v[64:128, HX + 1:HX + 2, 0:WX], x_in[s, :, 0:1, :])
                x2.append(v)

            def upconv_parity(wt, store, tensor_idx):
                """Parity-layout up-conv (c00): groups (a, c), N=512."""
                for s in range(B_LOC):
                    for k in range(NCH):
                        idx = s * NCH + k
                        pt = ps.tile([128, CPX], F32, tag="ps", bufs=3)
                        # group-inner order: the 4 col-groups issue adjacently
                        # so they stream through the PE array concurrently
                        for u in range(2):
                            for g in range(4):
                                a, c = g >> 1, g & 1
                                widx = a * 4 + c * 2 + u
                                rhs = x2[s][:, 8 * k + u + a:8 * k + u + a + 8,
                                            c:c + WX]
                                nc.tensor.matmul(
                                    pt[32 * g:32 * g + 32, :],
                                    wt[:, widx * 32:(widx + 1) * 32],
                                    rhs, start=(u == 0), stop=(u == 1),
                                    tile_position=(0, 32 * g))
                        _evac(pt, store, tensor_idx, idx)

            def upconv_il(wt, store, tensor_idx):
                """W-interleaved up-conv (c10): groups (a, row-half), N=256."""
                for s in range(B_LOC):
                    for k in range(NCH):
                        idx = s * NCH + k
                        pts = [ps.tile([128, CPX // 2], F32, tag=f"psh{c}",
                                       bufs=2, name=f"psh{c}")
                               for c in range(2)]
                        for c in range(2):
                            for u in range(2):
                                for g in range(4):
                                    a, ih = g >> 1, g & 1
                                    r0 = 8 * k + 4 * ih
                                    widx = a * 4 + c * 2 + u
                                    rhs = x2[s][:, r0 + u + a:r0 + u + a + 4,
                                                c:c + WX]
                                    nc.tensor.matmul(
                                        pts[c][32 * g:32 * g + 32, :],
                                        wt[:, widx * 32:(widx + 1) * 32],
                                        rhs, start=(u == 0), stop=(u == 1),
                                        tile_position=(0, 32 * g))
                        _evac2(pts[0], pts[1], store, tensor_idx, idx)

            upconv_parity(wu00, c00s, 0)
            stats_combine(0, ar1_in[:])
            nc.gpsimd.collective_compute(
                "AllReduce", ALU.add, replica_groups=[list(range(N_CORES))],
                ins=[ar1_in.opt()], outs=[ar1_out.opt()])
            nc.sync.dma_start(sg1[:], ar1_out[:])
            bn_coeffs(sg1[:], gbt[:, 0:1], gbt[:, 1:2], st0[:, 0:1], st0[:, 1:2])
            for g in range(4):
                nc.sync.dma_start(st0r[32 * g:32 * g + 32, :], st0[:])

            upconv_il(wu10, c10s, 1)  # overlaps AllReduce #1
            stats_combine(1, ar2_in[:, 0:2], nsum=2 * NTILE, nsq=NTILE)

        # ================= phase B: h = relu(BN0(c00)); conv01 ===============
        with (
            tc.tile_pool(name="h", bufs=2) as hp,
            tc.tile_pool(name="c01p", bufs=1) as c01p,
        ):
            c01s = c01p.tile([128, NTILE * CPX], BF16, tag="c01")
            for s in range(B_LOC):
                ht = hp.tile([128, HW2], BF16, tag="h")
                hv = ht[:].rearrange("p (r c) -> p r c", c=WX + 2)
                nc.gpsimd.memset(hv[:, :, 0:1], 0.0)
                nc.gpsimd.memset(hv[:, :, WX + 1:WX + 2], 0.0)
                for k in [NCH - 1] + list(range(NCH - 1)):
                    idx = s * NCH + k
                    src = c00s[:, idx * CPX:(idx + 1) * CPX].rearrange(
                        "p (r c) -> p r c", c=WX)
                    nc.scalar.activation(hv[:, 8 * k + 1:8 * k + 9, 1:WX + 1],
                                         src, ACT.Relu, scale=st0r[:, 0:1],
                                         bias=st0r[:, 1:2])
                # wrap halo rows (within each parity group)
                nc.vector.tensor_copy(hv[:, 0:1, :], hv[:, HX:HX + 1, :])
                nc.vector.tensor_copy(hv[:, HX + 1:HX + 2, :], hv[:, 1:2, :])

                for k in range(NCH):
                    idx = s * NCH + k
                    pts = [ps.tile([128, CPX // 2], F32, tag=f"psh{c}",
                                   bufs=2, name=f"psh{c}")
                           for c in range(2)]
                    for ga in range(2):
                        for d in range(2):
                            for e in range(2):
                                for m in range(4):
                                    al, ih = m >> 1, m & 1
                                    r0 = 8 * k + 4 * ih
                                    widx = al * 8 + ga * 4 + d * 2 + e
                                    rhs = hv[:, r0 + d + al:r0 + d + al + 4,
                                             e + ga:e + ga + WX]
                                    nc.tensor.matmul(
                                        pts[ga][32 * m:32 * m + 32, :],
                                        w01t[:, widx * 32:(widx + 1) * 32],
                                        rhs, start=(d == 0 and e == 0),
                                        stop=(d == 1 and e == 1),
                                        tile_position=(0, 32 * m))
                    _evac2(pts[0], pts[1], c01s, 2, idx)

            stats_combine(2, ar2_in[:, 2:4], nsum=2 * NTILE, nsq=NTILE)
            nc.gpsimd.collective_compute(
                "AllReduce", ALU.add, replica_groups=[list(range(N_CORES))],
                ins=[ar2_in.opt()], outs=[ar2_out.opt()])
            nc.sync.dma_start(sg2[:], ar2_out[:])

            # BN1 (c01, g01/b01) and BN2 (c10, g10/b10)
            s1, t1 = sm[:, 4:5], sm[:, 5:6]
            s2, t2 = sm[:, 6:7], sm[:, 7:8]
            bn_coeffs(sg2[:, 2:4], gbt[:, 2:3], gbt[:, 3:4], s1, t1)
            bn_coeffs(sg2[:, 0:2], gbt[:, 4:5], gbt[:, 5:6], s2, t2)
            rs1 = sm[:, 8:9]
            nc.vector.reciprocal(rs1, s1)
            nc.vector.tensor_mul(fin[:, 0:1], s2, rs1)   # r = s2/s1
            nc.vector.tensor_copy(fin[:, 1:2], s1)
            nc.vector.tensor_add(fin[:, 2:3], t1, t2)    # b' = t1 + t2
            for g in range(4):
                nc.sync.dma_start(finr[32 * g:32 * g + 32, :], fin[:])

            # ============ phase C: out = relu(s1*(c01 + r*c10) + b') =========
            # stores are W-interleaved: partition ch + 32*(2a + ih),
            # free (i 4, W 128); out row H = 2*(8k + 4*ih + i) + a.
            ov = out.rearrange("s ch (i2 a) w -> s a ch i2 w", a=2)
            for s in range(B_LOC):
                for k in range(NCH):
                    idx = s * NCH + k
                    tmp = scrap.tile([128, CPX], BF16, tag="fintmp")
                    nc.vector.scalar_tensor_tensor(
                        tmp[:], c10s[:, idx * CPX:(idx + 1) * CPX],
                        finr[:, 0:1], c01s[:, idx * CPX:(idx + 1) * CPX],
                        ALU.mult, ALU.add)
                    ot = scrap.tile([128, CPX], F32, tag="finout")
                    nc.scalar.activation(ot[:], tmp[:], ACT.Relu,
                                         scale=finr[:, 1:2], bias=finr[:, 2:3])
                    for g in range(4):
                        a, ih = g >> 1, g & 1
                        dst = ov[s, a][:, 8 * k + 4 * ih:8 * k + 4 * ih + 4, :]
                        src = ot[32 * g:32 * g + 32, :].rearrange(
                            "ch (i w) -> ch i w", w=2 * WX)
                        nc.sync.dma_start(dst, src)


def _build_nc(repeat=1):
    nc = bacc.Bacc("TRN2", target_bir_lowering=False, debug=False,
                   num_devices=N_CORES)
    x_in = nc.dram_tensor("x", [B_LOC, CIN, HX, WX], BF16,
                          kind="ExternalInput").ap()
    w00l = nc.dram_tensor("w00l", [8, 128, 32], BF16, kind="ExternalInput").ap()
    w10l = nc.dram_tensor("w10l", [8, 128, 32], BF16, kind="ExternalInput").ap()
    w01l = nc.dram_tensor("w01l", [16, 128, 32], BF16, kind="ExternalInput").ap()
    gb = nc.dram_tensor("gb", [6, 32], F32, kind="ExternalInput").ap()
    out = nc.dram_tensor("out", [B_LOC, COUT, 2 * HX, 2 * WX], F32,
                         kind="ExternalOutput").ap()
    with tile.TileContext(nc) as tc:
        for _ in range(repeat):
            _emit(nc, tc, x_in, w00l, w10l, w01l, gb, out)
    nc.compile()
    return nc


class _Runner:
    """Persistent jitted SPMD executor (mirrors bass2jax.run_bass_via_pjrt,
    but builds the jit once so steady-state calls skip tracing/compile)."""

    def __init__(self, nc):
        import jax
        from jax.sharding import Mesh, PartitionSpec
        from jax.experimental.shard_map import shard_map
        from concourse import bass2jax

        bass2jax.install_neuronx_cc_hook()
        self._jax = jax
        in_names, out_names, out_avals, zero_templates = [], [], [], []
        partition_name = (nc.partition_id_tensor.name
                          if nc.partition_id_tensor else None)
        for alloc in nc.m.functions[0].allocations:
            if not isinstance(alloc, mybir.MemoryLocationSet):
                continue
            name = alloc.memorylocations[0].name
            if alloc.kind == "ExternalInput":
                if name != partition_name:
                    in_names.append(name)
            elif alloc.kind == "ExternalOutput":
                out_names.append(name)
                shape = tuple(alloc.tensor_shape)
                dtype = mybir.dt.np(alloc.dtype)
                out_avals.append(jax.core.ShapedArray(shape, dtype))
                zero_templates.append((shape, dtype))
        self.in_names, self.out_names = in_names, out_names
        self.zero_templates = zero_templates
        self.out_avals = out_avals
        n_params, n_outs = len(in_names), len(out_names)
        all_names = list(in_names) + list(out_names)
        if partition_name is not None:
            all_names.append(partition_name)

        def _body(*args):
            operands = list(args)
            if partition_name is not None:
                operands.append(bass2jax.partition_id_tensor())
            outs = bass2jax._bass_exec_p.bind(
                *operands,
                out_avals=tuple(out_avals),
                in_names=tuple(all_names),
                out_names=tuple(out_names),
                lowering_input_output_aliases=(),
                sim_require_finite=True,
                sim_require_nnan=True,
                nc=nc,
            )
            return tuple(outs)

        devices = jax.devices()[:N_CORES]
        mesh = Mesh(np.asarray(devices), ("core",))
        donate = tuple(range(n_params, n_params + n_outs))
        self._fn = jax.jit(
            shard_map(_body, mesh=mesh,
                      in_specs=(PartitionSpec("core"),) * (n_params + n_outs),
                      out_specs=(PartitionSpec("core"),) * n_outs,
                      check_rep=False),
            donate_argnums=donate, keep_unused=True)

    def __call__(self, in_maps):
        concat_in = [np.concatenate([np.asarray(m[name]) for m in in_maps],
                                    axis=0) for name in self.in_names]
        zeros = [np.zeros((N_CORES * s[0],) + tuple(s[1:]), d)
                 for (s, d) in self.zero_templates]
        outs = self._fn(*concat_in, *zeros)
        return [
            {name: np.asarray(outs[i]).reshape(N_CORES, *self.out_avals[i].shape)[c]
             for i, name in enumerate(self.out_names)}
            for c in range(N_CORES)
        ]


_CACHE = {}


def _get_nc():
    if "nc" not in _CACHE:
        _CACHE["nc"] = _build_nc()
    return _CACHE["nc"]


def _make_in_maps(inputs):
    x = np.ascontiguousarray(
        np.asarray(inputs["x"], dtype=np.float32).astype(ml_dtypes.bfloat16))
    gb = np.stack([np.asarray(inputs[k], dtype=np.float32)
                   for k in ("g00", "b00", "g01", "b01", "g10", "b10")])
    w00l = _fold_up(np.asarray(inputs["w00"], dtype=np.float32)).astype(
        ml_dtypes.bfloat16)
    w10l = _fold_up(np.asarray(inputs["w10"], dtype=np.float32)).astype(
        ml_dtypes.bfloat16)
    w01l = _fold_c01(np.asarray(inputs["w01"], dtype=np.float32)).astype(
        ml_dtypes.bfloat16)
    return [{"x": x[i * B_LOC:(i + 1) * B_LOC],
             "w00l": w00l, "w10l": w10l, "w01l": w01l, "gb": gb}
            for i in range(N_CORES)]


def kernel(**inputs) -> np.ndarray:
    in_maps = _make_in_maps(inputs)
    if "runner" not in _CACHE:
        nc = _get_nc()
        # First call goes through the standard entry point (compiles the NEFF
        # and executes on cores 0-7); later calls reuse a cached jit so they
        # skip host-side retracing.
        res = bass_utils.run_bass_kernel_spmd(nc, in_maps,
                                              core_ids=list(range(N_CORES)))
        _CACHE["runner"] = _Runner(nc)
        return np.concatenate([r["out"] for r in res.results], axis=0)
    results = _CACHE["runner"](in_maps)
    return np.concatenate([r["out"] for r in results], axis=0)
